# revision 1
# baseline (speedup 1.0000x reference)
"""ChebyNet (K=4, 2-layer ChebConv + log_softmax) on 8 Trainium2 NeuronCores.

Strategy (1D node-parallel, per sharding hint):
  - Nodes are split contiguously across 8 cores (12500 each), then within a
    core sorted by in-degree (descending) so each 128-node gather block has a
    near-uniform slot count D_j (tight ELL, ~5% padding).
  - The symmetric normalization is folded into node features:
        prop(v) = -dis .* segsum(u[src]),   u = dis .* v
    so the sparse propagation is an UNWEIGHTED gather + segment-sum.
  - Per propagation, each core pulls u[src] rows for its edges from a local
    full-table copy via indirect DMA (ELL layout [128, D_j, F]) and reduces
    the D_j slot axis on the Vector engine.
  - The recurrence state u_k = dis .* Tx_k is exchanged with an 8-rank
    AllGather per propagation (5 total; the first table u0 = dis .* x is
    computed redundantly by every core from the full x input).
  - Dense projections sum_k Tx_k @ W_k accumulate in PSUM via
    PE-transpose(Tx block) + matmul.

kernel(**inputs) takes the FULL inputs and returns the FULL [N, C] output.
"""

import os
import sys

import numpy as np

for _p in ("/opt/trn_rl_repo",):
    if os.path.isdir(_p) and _p not in sys.path:
        sys.path.insert(0, _p)

from contextlib import ExitStack

import concourse.bacc as bacc
import concourse.mybir as mybir
import concourse.tile as tile
from concourse.bass import AP, IndirectOffsetOnAxis
from concourse.bass_utils import run_bass_kernel_spmd
from concourse.masks import make_identity

P = 128
NCORES = 8
F32 = mybir.dt.float32
I32 = mybir.dt.int32
ALU = mybir.AluOpType
AF = mybir.ActivationFunctionType
AX = mybir.AxisListType


# ---------------------------------------------------------------------------
# host-side graph partitioning / ELL construction (integer/structural only)
# ---------------------------------------------------------------------------

def preprocess(x, edge_index, W1, b1, W2, b2):
    N, F_IN = x.shape
    K, _, HID = W1.shape
    C_OUT = W2.shape[2]
    src = np.asarray(edge_index[0], dtype=np.int64)
    dst = np.asarray(edge_index[1], dtype=np.int64)
    E = src.shape[0]

    OWN = N // NCORES
    assert OWN * NCORES == N
    J = (OWN + P - 1) // P
    RPAD = P * J
    ZROW = NCORES * RPAD          # index of the shared zero row
    TROWS = ZROW + P              # table rows (multiple of 128)
    TCOLS = TROWS // P

    deg = np.bincount(dst, minlength=N).astype(np.int64)

    # per-core degree sort: rank 0 = highest degree
    rank_of = np.empty(N, np.int64)
    deg_rank = np.zeros((NCORES, RPAD), np.int64)
    for c in range(NCORES):
        dc = deg[c * OWN:(c + 1) * OWN]
        order = np.argsort(-dc, kind="stable")
        rank_of[c * OWN + order] = np.arange(OWN)
        deg_rank[c, :OWN] = dc[order]

    node_core = np.arange(N) // OWN
    p_of = rank_of % P
    j_of = rank_of // P
    # u-table row of each original node (grid layout: row = p*J + j)
    urow_of = node_core * RPAD + p_of * J + j_of

    # slots per block: max degree in block over all cores (same NEFF everywhere)
    degblk = deg_rank.reshape(NCORES, J, P)
    Dlist = np.maximum(degblk.max(axis=(0, 2)), 1).astype(np.int64)
    OFF = np.concatenate([[0], np.cumsum(Dlist)]).astype(np.int64)
    SUMD = int(OFF[-1])

    # edge -> (core, rank, slot)
    e_c = dst // OWN
    e_r = rank_of[dst]
    order_e = np.lexsort((urow_of[src], e_r, e_c))
    es = src[order_e]
    ec = e_c[order_e]
    er = e_r[order_e]
    gid = ec * RPAD + er
    counts = np.bincount(gid, minlength=NCORES * RPAD)
    starts = np.concatenate([[0], np.cumsum(counts)[:-1]])
    slot = np.arange(E) - starts[gid]
    ep = er % P
    ej = er // P
    col = OFF[ej] + slot
    idx_tab = np.full((NCORES, P, SUMD), ZROW, np.int32)
    idx_tab[ec, ep, col] = urow_of[es].astype(np.int32)

    x_perm = np.zeros((TROWS, F_IN), np.float32)
    x_perm[urow_of] = np.asarray(x, np.float32)
    deg_full = np.zeros((TROWS,), np.float32)
    deg_full[urow_of] = deg.astype(np.float32)
    deg_own = deg_full[:ZROW].reshape(NCORES, P, J)

    cfg = dict(
        N=N, E=E, F_IN=F_IN, HID=HID, C_OUT=C_OUT, K=K,
        OWN=OWN, J=J, RPAD=RPAD, ZROW=ZROW, TROWS=TROWS, TCOLS=TCOLS,
        Dlist=[int(d) for d in Dlist], OFF=[int(o) for o in OFF], SUMD=SUMD,
    )
    W2p = np.zeros((K, HID, 16), np.float32)
    W2p[:, :, :C_OUT] = np.asarray(W2, np.float32)
    b1r = np.broadcast_to(np.asarray(b1, np.float32), (P, HID)).copy()
    b2r = np.broadcast_to(np.asarray(b2, np.float32), (P, C_OUT)).copy()
    in_maps = []
    for c in range(NCORES):
        in_maps.append({
            "x_perm": x_perm,
            "xown": x_perm[c * RPAD:(c + 1) * RPAD].copy(),
            "deg_full": deg_full,
            "deg_own": np.ascontiguousarray(deg_own[c]),
            "idx_tab": np.ascontiguousarray(idx_tab[c]),
            "W1": np.asarray(W1, np.float32),
            "b1r": b1r,
            "W2": W2p,
            "b2r": b2r,
        })
    return cfg, in_maps, urow_of


# ---------------------------------------------------------------------------
# AP broadcast helpers (step-0 dims)
# ---------------------------------------------------------------------------

def _bcast_last(ap, n):
    """[P, c] -> [P, c, n] (innermost broadcast)."""
    return AP(ap.tensor, ap.offset, [*ap.ap, [0, n]])


def _bcast_mid(ap, reps):
    """[P, n] -> [P, reps, n] (middle broadcast)."""
    return AP(ap.tensor, ap.offset, [ap.ap[0], [0, reps], *ap.ap[1:]])


# ---------------------------------------------------------------------------
# device program
# ---------------------------------------------------------------------------

def build(cfg, debug=False, rep=1):
    J = cfg["J"]
    RPAD = cfg["RPAD"]
    ZROW = cfg["ZROW"]
    TROWS = cfg["TROWS"]
    TCOLS = cfg["TCOLS"]
    F_IN = cfg["F_IN"]
    HID = cfg["HID"]
    C_OUT = cfg["C_OUT"]
    K = cfg["K"]
    Dlist = cfg["Dlist"]
    OFF = cfg["OFF"]
    SUMD = cfg["SUMD"]
    DMAX = max(Dlist)
    SLOT = 16  # psum column slot per block (>= max(HID, C_OUT))
    assert HID <= SLOT and C_OUT <= SLOT and J * SLOT <= 2048

    nc = bacc.Bacc(
        "TRN2", target_bir_lowering=False, debug=False,
        enable_asserts=False, num_devices=NCORES,
    )

    x_in = nc.dram_tensor("x_perm", [TROWS, F_IN], F32, kind="ExternalInput")
    xown_in = nc.dram_tensor("xown", [RPAD, F_IN], F32, kind="ExternalInput")
    degf_in = nc.dram_tensor("deg_full", [TROWS], F32, kind="ExternalInput")
    dego_in = nc.dram_tensor("deg_own", [P, J], F32, kind="ExternalInput")
    idx_in = nc.dram_tensor("idx_tab", [P, SUMD], I32, kind="ExternalInput")
    w1_in = nc.dram_tensor("W1", [K, F_IN, HID], F32, kind="ExternalInput")
    b1_in = nc.dram_tensor("b1r", [P, HID], F32, kind="ExternalInput")
    w2_in = nc.dram_tensor("W2", [K, HID, SLOT], F32, kind="ExternalInput")
    b2_in = nc.dram_tensor("b2r", [P, C_OUT], F32, kind="ExternalInput")
    y_out = nc.dram_tensor("y", [RPAD, C_OUT], F32, kind="ExternalOutput")
    dbg = {}
    if debug:
        for nm, shp in [("dbg_u0", [TROWS, F_IN]), ("dbg_uf1", [TROWS, F_IN]),
                        ("dbg_g0", [P, 64 * F_IN]), ("dbg_z1", [P, J * F_IN]),
                        ("dbg_tx1", [P, J * F_IN]), ("dbg_acc1", [P, J * SLOT]),
                        ("dbg_h1", [P, J * HID])]:
            dbg[nm] = nc.dram_tensor(nm, shp, F32, kind="ExternalOutput")

    rg = [list(range(NCORES))]

    with ExitStack() as ctx:
        tc = ctx.enter_context(tile.TileContext(nc))
        dram = ctx.enter_context(tc.tile_pool(name="dram", bufs=1, space="DRAM"))
        cpool = ctx.enter_context(tc.tile_pool(name="const", bufs=1))

        # ---- constants ------------------------------------------------
        ident = cpool.tile([P, P], F32)
        make_identity(nc, ident)
        idx_sb = cpool.tile([P, SUMD], I32)
        nc.sync.dma_start(out=idx_sb, in_=idx_in.ap())
        w1_sb = cpool.tile([F_IN, K * HID], F32)
        for k in range(K):
            nc.sync.dma_start(out=w1_sb[:, k * HID:(k + 1) * HID], in_=w1_in.ap()[k])
        w2_sb = cpool.tile([HID, K * SLOT], F32)
        for k in range(K):
            nc.sync.dma_start(out=w2_sb[:, k * SLOT:(k + 1) * SLOT], in_=w2_in.ap()[k])
        b1_sb = cpool.tile([P, HID], F32)
        nc.sync.dma_start(out=b1_sb, in_=b1_in.ap())
        b2_sb = cpool.tile([P, C_OUT], F32)
        nc.sync.dma_start(out=b2_sb, in_=b2_in.ap())
        zrow_sb = cpool.tile([P, F_IN], F32)
        nc.vector.memset(zrow_sb, 0.0)

        work = ctx.enter_context(tc.tile_pool(name="work", bufs=1))
        txp = ctx.enter_context(tc.tile_pool(name="txp", bufs=3))
        zp = ctx.enter_context(tc.tile_pool(name="zp", bufs=2))
        gp = ctx.enter_context(tc.tile_pool(name="gp", bufs=4))
        lp = ctx.enter_context(tc.tile_pool(name="lp", bufs=4))
        pp = ctx.enter_context(tc.tile_pool(name="pp", bufs=2, space="PSUM"))
        ap_ = ctx.enter_context(tc.tile_pool(name="acc", bufs=1, space="PSUM"))

        yv = y_out.ap().rearrange("(p j) c -> p j c", p=P)

        def body(r):
            tag = f"r{r}" if rep > 1 else ""
            dis_e = cpool.tile([P, J, F_IN], F32, tag="dis_e", name=f"dis_e{r}")
            ndis_e = cpool.tile([P, J, F_IN], F32, tag="ndis_e",
                                name=f"ndis_e{r}")
            u0_full = dram.tile([TROWS, F_IN], F32, tag="u0", name=f"u0_{r}")

            # ---- own-node dis + expanded copies ------------------------
            dego_sb = work.tile([P, J], F32, tag="dego", name=f"dego{r}")
            nc.sync.dma_start(out=dego_sb, in_=dego_in.ap())
            dtmp = work.tile([P, J], F32, tag="dtmp", name=f"dtmp{r}")
            nc.vector.tensor_scalar(dtmp, dego_sb, 1.0, None, ALU.max)
            dsq = work.tile([P, J], F32, tag="dsq", name=f"dsq{r}")
            nc.scalar.activation(dsq, dtmp, AF.Sqrt)
            drs = work.tile([P, J], F32, tag="drs", name=f"drs{r}")
            nc.vector.reciprocal(drs, dsq)
            dmask = work.tile([P, J], F32, tag="dmask", name=f"dmask{r}")
            nc.vector.tensor_scalar(dmask, dego_sb, 0.0, None, ALU.is_gt)
            dis = work.tile([P, J], F32, tag="dis", name=f"dis{r}")
            nc.vector.tensor_mul(dis, drs, dmask)
            for j in range(J):
                nc.vector.tensor_copy(
                    out=dis_e[:, j, :],
                    in_=dis[:, j:j + 1].to_broadcast([P, F_IN]))
            nc.vector.tensor_scalar(ndis_e, dis_e, -1.0, None, ALU.mult)

            # ---- u0 = dis .* x for the WHOLE table ---------------------
            xv = x_in.ap().rearrange("(p t) f -> p t f", p=P)
            dv = degf_in.ap().rearrange("(p t) -> p t", p=P)
            u0v = u0_full.rearrange("(p t) f -> p t f", p=P)
            CH = 64
            for t0 in range(0, TCOLS, CH):
                n = min(CH, TCOLS - t0)
                xa = work.tile([P, CH, F_IN], F32, tag="xa", bufs=2,
                               name=f"xa{r}_{t0}")
                nc.sync.dma_start(out=xa[:, :n, :], in_=xv[:, t0:t0 + n, :])
                da = work.tile([P, CH], F32, tag="da", bufs=2,
                               name=f"da{r}_{t0}")
                nc.sync.dma_start(out=da[:, :n], in_=dv[:, t0:t0 + n])
                ta = work.tile([P, CH], F32, tag="ta", bufs=2,
                               name=f"ta{r}_{t0}")
                nc.vector.tensor_scalar(ta[:, :n], da[:, :n], 1.0, None,
                                        ALU.max)
                sa = work.tile([P, CH], F32, tag="sa", bufs=2,
                               name=f"sa{r}_{t0}")
                nc.scalar.activation(sa[:, :n], ta[:, :n], AF.Sqrt)
                ra = work.tile([P, CH], F32, tag="ra", bufs=2,
                               name=f"ra{r}_{t0}")
                nc.vector.reciprocal(ra[:, :n], sa[:, :n])
                ma = work.tile([P, CH], F32, tag="ma", bufs=2,
                               name=f"ma{r}_{t0}")
                nc.vector.tensor_scalar(ma[:, :n], da[:, :n], 0.0, None,
                                        ALU.is_gt)
                fa = work.tile([P, CH], F32, tag="fa", bufs=2,
                               name=f"fa{r}_{t0}")
                nc.vector.tensor_mul(fa[:, :n], ra[:, :n], ma[:, :n])
                ua = work.tile([P, CH, F_IN], F32, tag="ua", bufs=2,
                               name=f"ua{r}_{t0}")
                nc.vector.tensor_mul(
                    ua[:, :n, :], xa[:, :n, :], _bcast_last(fa[:, :n], F_IN))
                nc.sync.dma_start(out=u0v[:, t0:t0 + n, :], in_=ua[:, :n, :])

            if debug:
                nc.sync.dma_start(out=dbg["dbg_u0"].ap(), in_=u0_full)

            # ---- helpers ----------------------------------------------
            def gather_prop(u_dram, F, zname):
                z = zp.tile([P, J, F_IN], F32, tag="z", name=zname)
                for j in range(J):
                    Dj = Dlist[j]
                    g = gp.tile([P, DMAX, F_IN], F32, tag="g",
                                name=f"{zname}_g{j}")
                    for d in range(Dj):
                        col = OFF[j] + d
                        nc.gpsimd.indirect_dma_start(
                            out=g[:, d, :F],
                            out_offset=None,
                            in_=u_dram,
                            in_offset=IndirectOffsetOnAxis(
                                ap=idx_sb[:, col:col + 1], axis=0),
                        )
                    if debug and zname == "l1_z1" and j == 0:
                        nc.sync.dma_start(
                            out=dbg["dbg_g0"].ap()[:, :Dj * F],
                            in_=g[:, :Dj, :F])
                    nc.vector.tensor_reduce(
                        out=z[:, j, :F],
                        in_=g[:, :Dj, :F].rearrange("p d f -> p f d"),
                        axis=AX.X, op=ALU.add)
                if debug and zname == "l1_z1":
                    nc.sync.dma_start(
                        out=dbg["dbg_z1"].ap().rearrange(
                            "p (j f) -> p j f", j=J),
                        in_=z)
                return z

            def proj(tx, w_sb, k, osum, FI, pname):
                ps = ap_.tile([P, J * SLOT], F32, space="PSUM", tag="acc",
                              bufs=1, name=f"{pname}_ps")
                for j in range(J):
                    tp = pp.tile([F_IN, P], F32, space="PSUM", tag="tp",
                                 name=f"{pname}_tp{j}")
                    nc.tensor.transpose(out=tp[:FI, :], in_=tx[:, j, :FI],
                                        identity=ident)
                    lh = lp.tile([F_IN, P], F32, tag="lh",
                                 name=f"{pname}_lh{j}")
                    nc.scalar.copy(out=lh[:FI, :], in_=tp[:FI, :])
                    nc.tensor.matmul(
                        out=ps[:, j * SLOT:(j + 1) * SLOT],
                        lhsT=lh[:FI, :],
                        rhs=w_sb[:, k * SLOT:(k + 1) * SLOT],
                        start=True, stop=True)
                if k == 0:
                    nc.vector.tensor_copy(out=osum, in_=ps)
                else:
                    nc.vector.tensor_add(out=osum, in0=osum, in1=ps)

            def allgather(uown_dram, F, tag2, name):
                uf = dram.tile([TROWS, F], F32, tag=tag2, name=name)
                nc.sync.dma_start(out=uf[ZROW:TROWS, :], in_=zrow_sb[:, :F])
                nc.gpsimd.collective_compute(
                    "AllGather", ALU.bypass, replica_groups=rg,
                    ins=[uown_dram.opt()], outs=[uf[0:ZROW, :].opt()])
                return uf

            def store_uown(u_sb, F, name):
                uo = dram.tile([RPAD, F], F32, tag="uown", name=name)
                nc.sync.dma_start(
                    out=uo.rearrange("(p j) f -> p j f", p=P),
                    in_=u_sb[:, :, :F])
                return uo

            def layer(tx0, w_sb, u_first_full, FI, lname):
                acc = cpool.tile([P, J * SLOT], F32, tag="osum", bufs=1,
                                 name=f"{lname}_acc{r}")
                proj(tx0, w_sb, 0, acc, FI, f"{lname}p0_{r}")
                txs = [tx0]
                ufull = u_first_full
                for k in range(1, K):
                    z = gather_prop(ufull, FI, f"{lname}_z{k}_{r}")
                    txk = txp.tile([P, J, F_IN], F32, tag="tx",
                                   name=f"{lname}_tx{k}_{r}")
                    nc.vector.tensor_mul(
                        txk[:, :, :FI], ndis_e[:, :, :FI], z[:, :, :FI])
                    if debug and lname == "l1" and k == 1:
                        nc.sync.dma_start(
                            out=dbg["dbg_tx1"].ap().rearrange(
                                "p (j f) -> p j f", j=J),
                            in_=txk)
                    if k > 1:
                        nc.vector.tensor_scalar(
                            txk[:, :, :FI], txk[:, :, :FI], 2.0, None,
                            ALU.mult)
                        nc.vector.tensor_sub(
                            txk[:, :, :FI], txk[:, :, :FI],
                            txs[k - 2][:, :, :FI])
                    proj(txk, w_sb, k, acc, FI, f"{lname}p{k}_{r}")
                    txs.append(txk)
                    if k < K - 1:
                        un = txp.tile([P, J, F_IN], F32, tag="un", bufs=1,
                                      name=f"{lname}_u{k}_{r}")
                        nc.vector.tensor_mul(
                            un[:, :, :FI], dis_e[:, :, :FI], txk[:, :, :FI])
                        uo = store_uown(un, FI, f"{lname}_uo{k}_{r}")
                        ufull = allgather(uo, FI, f"ufull_{lname}",
                                          f"{lname}_uf{k}_{r}")
                        if debug and lname == "l1" and k == 1:
                            nc.sync.dma_start(
                                out=dbg["dbg_uf1"].ap()[:, :FI], in_=ufull)
                if debug and lname == "l1":
                    nc.sync.dma_start(out=dbg["dbg_acc1"].ap(), in_=acc)
                return acc.rearrange("p (j s) -> p j s", s=SLOT)

            # ---- layer 1 ----------------------------------------------
            tx0 = txp.tile([P, J, F_IN], F32, tag="tx", name=f"tx0_{r}")
            nc.sync.dma_start(
                out=tx0, in_=xown_in.ap().rearrange("(p j) f -> p j f", p=P))
            acc1 = layer(tx0, w1_sb, u0_full, F_IN, "l1")
            h1 = cpool.tile([P, J, HID], F32, tag="h1", name=f"h1_{r}")
            nc.vector.tensor_add(h1, acc1[:, :, :HID], _bcast_mid(b1_sb, J))
            nc.scalar.activation(h1, h1, AF.Relu)
            if debug:
                nc.sync.dma_start(
                    out=dbg["dbg_h1"].ap().rearrange("p (j f) -> p j f", j=J),
                    in_=h1)

            # ---- layer 2 ----------------------------------------------
            v0 = txp.tile([P, J, F_IN], F32, tag="tx", name=f"l2_v0_{r}")
            nc.vector.tensor_copy(v0[:, :, :HID], h1)
            un0 = txp.tile([P, J, F_IN], F32, tag="un", bufs=1,
                           name=f"l2_u0_{r}")
            nc.vector.tensor_mul(un0[:, :, :HID], dis_e[:, :, :HID], h1)
            uo0 = store_uown(un0, HID, f"l2_uo0_{r}")
            w0_full = allgather(uo0, HID, "ufull_l2", f"l2_uf0_{r}")
            acc2 = layer(v0, w2_sb, w0_full, HID, "l2")

            # ---- log_softmax ------------------------------------------
            s2 = work.tile([P, J, C_OUT], F32, tag="s2", name=f"s2_{r}")
            nc.vector.tensor_add(s2, acc2[:, :, :C_OUT], _bcast_mid(b2_sb, J))
            mx = work.tile([P, J], F32, tag="mx", name=f"mx_{r}")
            nc.vector.tensor_reduce(out=mx, in_=s2, axis=AX.X, op=ALU.max)
            sh = work.tile([P, J, C_OUT], F32, tag="sh", name=f"sh_{r}")
            nc.vector.tensor_tensor(
                out=sh, in0=s2, in1=_bcast_last(mx, C_OUT), op=ALU.subtract)
            ex = work.tile([P, J, C_OUT], F32, tag="ex", name=f"ex_{r}")
            nc.scalar.activation(ex, sh, AF.Exp)
            ssum = work.tile([P, J], F32, tag="ssum", name=f"ssum_{r}")
            nc.vector.tensor_reduce(out=ssum, in_=ex, axis=AX.X, op=ALU.add)
            lg = work.tile([P, J], F32, tag="lg", name=f"lg_{r}")
            nc.scalar.activation(lg, ssum, AF.Ln)
            yt = work.tile([P, J, C_OUT], F32, tag="ex", name=f"yt_{r}")
            nc.vector.tensor_tensor(
                out=yt, in0=sh, in1=_bcast_last(lg, C_OUT), op=ALU.subtract)
            nc.sync.dma_start(out=yv, in_=yt)

        for r in range(rep):
            body(r)

    nc.compile()
    return nc


# ---------------------------------------------------------------------------
# entry point
# ---------------------------------------------------------------------------

_LAST_PERF = {}


def kernel(x, edge_index, W1, b1, W2, b2):
    cfg, in_maps, urow_of = preprocess(x, edge_index, W1, b1, W2, b2)
    nc = build(cfg)
    trace = bool(int(os.environ.get("GNN_TRACE", "0")))
    res = run_bass_kernel_spmd(
        nc, in_maps, core_ids=list(range(NCORES)), trace=trace)
    _LAST_PERF.clear()
    _LAST_PERF.update(
        exec_time_ns=res.exec_time_ns,
        mean_exec_time_ns=res.mean_exec_time_ns,
        trace=res.instructions_and_trace[1] if res.instructions_and_trace else None,
    )
    full_y = np.concatenate([res.results[c]["y"] for c in range(NCORES)], axis=0)
    return np.ascontiguousarray(full_y[urow_of]).astype(np.float32)

